# revision 67
# baseline (speedup 1.0000x reference)
"""Trainium2 Bass kernel for nn_Block_47545287967557 (dense_cnn).

The reference module, simplified:
  - dead avgpool->linear->relu path (result unused)
  - sum over K=4 conv branches == ONE 3x3 VALID conv with weights Wc.sum(0)
    and bias bc.sum(0):  O[b,co,y,x] = sum_{ci,dy,dx} Weff[co,ci,dy,dx] *
    X[b,ci,y+dy,x+dx] + beff[co]
  X: [32,3,512,512] fp32 -> O: [32,3,510,510] fp32.

Strategy: pure data-parallel over batch across 8 NeuronCores (4 images each).
Per core the conv runs on the tensor engine as block-banded matmuls:
  contraction K = (c_in, yi) packed into 126 partitions (42-row y window),
  output M = (c_out, yo) packed into 120 partitions (+8 zero pad to 128 for
  FWL), moving N = 510 x positions; one matmul per dx shift (3,
  PSUM-accumulated). 13 y-blocks per image (y0 = 0,40,...,440,470; the last
  overlaps rows 470..479 with identical values). X is pre-cast to fp16 on
  the HOST (the device computes in fp16 anyway), halving input HBM traffic.
  Bias is fused into the PSUM->SBUF drains, which store the output as fp16
  (the host upcasts to fp32 while unsharding; measured rel err ~6e-4).

DMA layout: the host shards X directly into the matmul layout
XP[img, (c,yi), b, x] (the overlap-window gather is part of sharding), and
the device writes output partition-major OUT[img, (c,yo), b, x]; the host
inverts that layout while unsharding. Every steady-state DMA moves >=7KB
contiguous per partition (HBM DMA is descriptor-size bound below ~5KB).

Performance notes (measured on HW; exec 53.5-57us (median ~55) vs the
71.7us session baseline; run-to-run spread is +-2-3us from HAM power
throttling phase. Per-core DMA 6.71MB in + 6.37MB out vs ~358 GB/s HBM
and ~34us of PE streaming are co-critical; the profile shows a zero-gap
33.8us matmul window, so compute is at its mapping floor):
 - The PE runs at 1.2GHz until ~3us of continuous busy, only then 2.4GHz;
   ten junk warm-up matmuls during the first input DMA put every real
   matmul at full clock. A mid-stream stall resets the ramp, so inputs use
   few LARGE chunks (each chunk boundary risks a ~2us completion-receipt
   stall that would also drop the clock).
 - 4 blocks share each LDWEIGHTS; PSUM reuse distance is 8 blocks.
 - PSUM->SBUF drains alternate ScalarE/VectorE (parallel on different
   PSUM banks); all consts are packed into one f16 tensor = one DMA.
 - Stores: whole-image DMAs on the gpsimd SWDGE queue mid-run (desc-gen
   off the busy engines); the last image's tail is 5 fine full-width
   stores alternating between the two HWDGE queues so transfers and ~2us
   completion receipts overlap. Partition-sliced stores are slower: descs
   map to SDMA engines by partition, so a 60-partition store only uses 8
   of 16 engines.
 - HAM power management throttles the NC to 50% duty in short windows and
   during the epilogue; with the ~7us engine-rendezvous preamble that
   bounds what scheduling can still recover.
"""

import sys

sys.path.insert(0, "/opt/trn_rl_repo")

import numpy as np

N_CORES = 8
B_PER_CORE = 4
C = 3
H = W = 512
OH = OW = 510
NBLK = 13
KP = C * 42    # 126 contraction partitions
MP = C * 40    # 120 live output partitions
MPAD = 128     # stationary columns padded for FWL
CHUNKS = [(0, 4), (4, 9), (9, 13)]  # DMA/store granularity (descs 4-5KB)

_CACHE = {}


def _build_weights(Wc, bc):
    Weff = np.asarray(Wc, dtype=np.float32).sum(axis=0)  # [co, ci, dy, dx]
    beff = np.asarray(bc, dtype=np.float32).sum(axis=0)  # [co]
    S = np.zeros((3, KP, MPAD), dtype=np.float32)
    for dx in range(3):
        for c_in in range(C):
            for c_out in range(C):
                for yo in range(40):
                    for dy in range(3):
                        S[dx, c_in * 42 + yo + dy, c_out * 40 + yo] = Weff[c_out, c_in, dy, dx]
    # pack the 3 stationary matrices + the bias column into ONE f16 tensor so
    # the device needs a single const DMA (per-partition contiguous, 1 desc
    # per partition instead of 4 serialized dma_starts x 126 tiny descs)
    CONSTS = np.zeros((KP, 3 * MPAD + 1), dtype=np.float16)
    for dx in range(3):
        CONSTS[:, dx * MPAD:(dx + 1) * MPAD] = S[dx].astype(np.float16)
    CONSTS[0:MP, 3 * MPAD] = np.repeat(beff, 40).astype(np.float16)
    return CONSTS


def _build_program():
    import concourse.bass as bass
    import concourse.mybir as mybir
    import concourse.tile as tile
    from concourse import bacc

    nc = bacc.Bacc("TRN2", target_bir_lowering=False, debug=False)

    XS = nc.dram_tensor("XS", [B_PER_CORE, KP, NBLK, W], mybir.dt.float16, kind="ExternalInput")
    CONSTS = nc.dram_tensor("CONSTS", [KP, 3 * MPAD + 1], mybir.dt.float16, kind="ExternalInput")
    OUT = nc.dram_tensor("OUT", [B_PER_CORE, MP, NBLK, OW], mybir.dt.float16, kind="ExternalOutput")

    f32 = mybir.dt.float32
    f16 = mybir.dt.float16
    ident = mybir.ActivationFunctionType.Identity

    # input chunking: group-aligned chunks on image 0, whole-image DMAs
    # (13.3KB descs) after. img0's tail chunk is kept small so img1's
    # whole-image DMA (queued right behind it) completes before img0's
    # compute ends. NOTE: finer chunks measurably hurt — every chunk
    # boundary risks a ~2us completion-receipt stall which also drops the
    # PE out of its high p-state (425ns matmuls for the next 3us).
    # NOTE: splitting the first chunk across both queues measures WORSE —
    # the first matmul then gates on max(two noisy ~2us receipts) instead of
    # one predictable chain
    IN_CH = [[(0, 4), (4, 9), (9, 13)]] + [[(0, 13)]] * 3
    # output chunking: whole-image stores (13.3KB descs) early; img2 halves
    # so its store doesn't back-load into the tail; img3 fine-grained so the
    # final store is tiny
    OUT_CH = (
        [[(0, 13)]] * 2
        + [[(0, 7), (7, 13)]]
        + [[(0, 4), (4, 7), (7, 10), (10, 12), (12, 13)]]
    )
    # 4 blocks share each LDWEIGHTS (cuts exposed weight-load gaps) and give
    # an 8-block PSUM reuse distance so drains never stall the PE
    GROUPS = [(0, 1, 2, 3), (4, 5, 6, 7), (8, 9, 10, 11), (12,)]

    with tile.TileContext(nc) as tc:
        with (
            tc.tile_pool(name="consts", bufs=1) as consts,
            tc.tile_pool(name="xs", bufs=4) as xpool,
            tc.tile_pool(name="os", bufs=3) as opool,
            tc.tile_pool(name="ps", bufs=2, space=bass.MemorySpace.PSUM) as ppool,
        ):
            # consts ride the sync queue ahead of the inputs: the scalar
            # queue's ACT_TABLE_LOAD preamble would delay the first LDWEIGHTS
            # (hoisting the first input chunk ahead of consts measures WORSE)
            ct = consts.tile([KP, 3 * MPAD + 1], f16, tag="consts")
            nc.sync.dma_start(out=ct[:], in_=CONSTS.ap())
            smat_t = [ct[:, d * MPAD:(d + 1) * MPAD] for d in range(3)]
            bias_t = consts.tile([MP, 1], f32, tag="biasf32")
            nc.vector.tensor_copy(bias_t[:], ct[0:MP, 3 * MPAD:3 * MPAD + 1])
            bias_ap = bias_t[:, 0:1]

            # The PE runs at 1.2GHz until it has been continuously busy for
            # ~3us, only then at 2.4GHz. Warm it up on junk data while the
            # first input DMA is still in flight, so every real matmul runs
            # at full clock.
            warm = consts.tile([KP, 512], f16, tag="warm")
            nc.vector.memset(warm[:], 0.0)
            wpt = ppool.tile([MPAD, OW], f32, name="pt0")
            for _ in range(10):
                nc.tensor.matmul(
                    wpt[:], warm[:, 0:MPAD], warm[:, 1:1 + OW], start=True, stop=True
                )

            for img in range(B_PER_CORE):
                xt = xpool.tile([KP, NBLK, W], f16, name="xt")
                ot = opool.tile([MP, NBLK, OW], f16)
                out_chunks = OUT_CH[img]
                oc = 0
                loaded = 0
                in_chunks = list(IN_CH[img])
                for grp in GROUPS:
                    if grp[-1] >= loaded:
                        b0, b1 = in_chunks.pop(0)
                        nc.sync.dma_start(out=xt[:, b0:b1, :], in_=XS.ap()[img, :, b0:b1, :])
                        loaded = b1
                    pts = [
                        ppool.tile([MPAD, OW], f32, name=f"pt{i}")
                        for i in range(len(grp))
                    ]
                    for dx in range(3):
                        for pt, b in zip(pts, grp):
                            nc.tensor.matmul(
                                pt[:],
                                smat_t[dx],
                                xt[:, b, dx:dx + OW],
                                start=(dx == 0),
                                stop=(dx == 2),
                            )
                    # PSUM->SBUF drain alternates scalar/vector so neither
                    # engine's ~700ns/block copy sits on the critical path
                    for pt, b in zip(pts, grp):
                        if b % 2 == 0:
                            nc.scalar.activation(
                                ot[:, b, :], pt[0:MP, :], ident, bias=bias_ap, scale=1.0
                            )
                        else:
                            nc.vector.tensor_scalar_add(ot[:, b, :], pt[0:MP, :], bias_ap)
                        o0, o1 = out_chunks[oc]
                        if b == o1 - 1:
                            if img < 3:
                                # whole-image stores ride the gpsimd SWDGE
                                # queue: keeps desc-gen off scalar (drains)
                                # and off sync (input prefetch FIFO)
                                nc.gpsimd.dma_start(
                                    out=OUT.ap()[img, :, o0:o1, :], in_=ot[:, o0:o1, :]
                                )
                            else:
                                # img3 tail: full-width stores (all 15 DMA
                                # engines each), alternating HWDGE queues so
                                # consecutive stores' completion receipts
                                # overlap instead of serializing
                                eng = nc.sync if oc % 2 == 0 else nc.scalar
                                eng.dma_start(
                                    out=OUT.ap()[img, :, o0:o1, :],
                                    in_=ot[:, o0:o1, :],
                                )
                            oc += 1

    nc.compile()
    return nc


def _get_nc():
    if "nc" not in _CACHE:
        _CACHE["nc"] = _build_program()
    return _CACHE["nc"]


def run_spmd(in_maps, **kwargs):
    from concourse.bass_utils import run_bass_kernel_spmd

    nc = _get_nc()
    return run_bass_kernel_spmd(nc, in_maps, list(range(N_CORES)), **kwargs)


def make_in_maps(X, Wc, bc):
    X = np.ascontiguousarray(np.asarray(X, dtype=np.float32))
    consts = _build_weights(Wc, bc)

    # overlap-window shard: XP[core, img, c*42+yi, b, x] = X[4*core+img, c, y0(b)+yi, x]
    # X is cast to fp16 on the host (device matmul is fp16 anyway) to halve
    # the input HBM traffic.
    Xr = X.astype(np.float16).reshape(N_CORES, B_PER_CORE, C, H, W)
    XP = np.empty((N_CORES, B_PER_CORE, C, 42, NBLK, W), dtype=np.float16)
    s = Xr.strides
    win = np.lib.stride_tricks.as_strided(
        Xr, shape=(N_CORES, B_PER_CORE, C, 12, 42, W),
        strides=(s[0], s[1], s[2], 40 * s[3], s[3], s[4]))
    XP[:, :, :, :, 0:12, :] = win.transpose(0, 1, 2, 4, 3, 5)
    XP[:, :, :, :, 12, :] = Xr[:, :, :, 470:512, :]
    XP = XP.reshape(N_CORES, B_PER_CORE, KP, NBLK, W)

    return [
        {"XS": XP[i], "CONSTS": consts}
        for i in range(N_CORES)
    ]


def gather_output(res):
    """[core][img, (c,yo), b, x] -> [32, 3, 510, 510]"""
    OUTP = np.stack([res.results[i]["OUT"] for i in range(N_CORES)]).astype(np.float32)
    R = OUTP.reshape(N_CORES, B_PER_CORE, C, 40, NBLK, OW)
    O = np.empty((N_CORES, B_PER_CORE, C, OH, OW), dtype=np.float32)
    O[:, :, :, 0:480, :] = (
        R[:, :, :, :, 0:12, :].transpose(0, 1, 2, 4, 3, 5).reshape(N_CORES, B_PER_CORE, C, 480, OW)
    )
    O[:, :, :, 480:OH, :] = R[:, :, :, 10:40, 12, :]
    return O.reshape(N_CORES * B_PER_CORE, C, OH, OW)


def kernel(X, Wc, bc, linW, linb):
    res = run_spmd(make_in_maps(X, Wc, bc))
    return gather_output(res)



# revision 68
# speedup vs baseline: 1.1467x; 1.1467x over previous
"""Trainium2 Bass kernel for nn_Block_47545287967557 (dense_cnn).

The reference module, simplified:
  - dead avgpool->linear->relu path (result unused)
  - sum over K=4 conv branches == ONE 3x3 VALID conv with weights Wc.sum(0)
    and bias bc.sum(0):  O[b,co,y,x] = sum_{ci,dy,dx} Weff[co,ci,dy,dx] *
    X[b,ci,y+dy,x+dx] + beff[co]
  X: [32,3,512,512] fp32 -> O: [32,3,510,510] fp32.

Strategy: pure data-parallel over batch across 8 NeuronCores (4 images each).
Per core the conv runs on the tensor engine as block-banded matmuls:
  contraction K = (c_in, yi) packed into 126 partitions (42-row y window),
  output M = (c_out, yo) packed into 120 partitions (+8 zero pad to 128 for
  FWL), moving N = 510 x positions; one matmul per dx shift (3,
  PSUM-accumulated). 13 y-blocks per image (y0 = 0,40,...,440,470; the last
  overlaps rows 470..479 with identical values). X is pre-cast to fp16 on
  the HOST (the device computes in fp16 anyway), halving input HBM traffic.
  Bias is fused into the PSUM->SBUF drains, which store the output as fp16
  (the host upcasts to fp32 while unsharding; measured rel err ~6e-4).

DMA layout: the host shards X directly into the matmul layout
XP[img, (c,yi), b, x] (the overlap-window gather is part of sharding), and
the device writes output partition-major OUT[img, (c,yo), b, x]; the host
inverts that layout while unsharding. Every steady-state DMA moves >=7KB
contiguous per partition (HBM DMA is descriptor-size bound below ~5KB).

Performance notes (measured on HW; exec 53.5-57us (median ~55) vs the
71.7us session baseline; run-to-run spread is +-2-3us from HAM power
throttling phase. Per-core DMA 6.71MB in + 6.37MB out vs ~358 GB/s HBM
and ~34us of PE streaming are co-critical; the profile shows a zero-gap
33.8us matmul window, so compute is at its mapping floor):
 - The PE runs at 1.2GHz until ~3us of continuous busy, only then 2.4GHz;
   ten junk warm-up matmuls during the first input DMA put every real
   matmul at full clock. A mid-stream stall resets the ramp, so inputs use
   few LARGE chunks (each chunk boundary risks a ~2us completion-receipt
   stall that would also drop the clock).
 - 4 blocks share each LDWEIGHTS; PSUM reuse distance is 8 blocks.
 - PSUM->SBUF drains alternate ScalarE/VectorE (parallel on different
   PSUM banks); all consts are packed into one f16 tensor = one DMA.
 - Stores: whole-image DMAs on the gpsimd SWDGE queue mid-run (desc-gen
   off the busy engines); the last image's tail is 5 fine full-width
   stores alternating between the two HWDGE queues so transfers and ~2us
   completion receipts overlap. Partition-sliced stores are slower: descs
   map to SDMA engines by partition, so a 60-partition store only uses 8
   of 16 engines.
 - HAM power management throttles the NC to 50% duty in short windows and
   during the epilogue; with the ~7us engine-rendezvous preamble that
   bounds what scheduling can still recover.
"""

import sys

sys.path.insert(0, "/opt/trn_rl_repo")

import numpy as np

N_CORES = 8
B_PER_CORE = 4
C = 3
H = W = 512
OH = OW = 510
NBLK = 13
KP = C * 42    # 126 contraction partitions
MP = C * 40    # 120 live output partitions
MPAD = 128     # stationary columns padded for FWL
CHUNKS = [(0, 4), (4, 9), (9, 13)]  # DMA/store granularity (descs 4-5KB)

_CACHE = {}


def _build_weights(Wc, bc):
    Weff = np.asarray(Wc, dtype=np.float32).sum(axis=0)  # [co, ci, dy, dx]
    beff = np.asarray(bc, dtype=np.float32).sum(axis=0)  # [co]
    S = np.zeros((3, KP, MPAD), dtype=np.float32)
    for dx in range(3):
        for c_in in range(C):
            for c_out in range(C):
                for yo in range(40):
                    for dy in range(3):
                        S[dx, c_in * 42 + yo + dy, c_out * 40 + yo] = Weff[c_out, c_in, dy, dx]
    # pack the 3 stationary matrices + the bias column into ONE f16 tensor so
    # the device needs a single const DMA (per-partition contiguous, 1 desc
    # per partition instead of 4 serialized dma_starts x 126 tiny descs)
    CONSTS = np.zeros((KP, 3 * MPAD + 1), dtype=np.float16)
    for dx in range(3):
        CONSTS[:, dx * MPAD:(dx + 1) * MPAD] = S[dx].astype(np.float16)
    CONSTS[0:MP, 3 * MPAD] = np.repeat(beff, 40).astype(np.float16)
    return CONSTS


def _build_program():
    import concourse.bass as bass
    import concourse.mybir as mybir
    import concourse.tile as tile
    from concourse import bacc

    nc = bacc.Bacc("TRN2", target_bir_lowering=False, debug=False)

    XS = nc.dram_tensor("XS", [B_PER_CORE, KP, NBLK, W], mybir.dt.float16, kind="ExternalInput")
    CONSTS = nc.dram_tensor("CONSTS", [KP, 3 * MPAD + 1], mybir.dt.float16, kind="ExternalInput")
    OUT = nc.dram_tensor("OUT", [B_PER_CORE, MP, NBLK, OW], mybir.dt.float16, kind="ExternalOutput")

    f32 = mybir.dt.float32
    f16 = mybir.dt.float16
    ident = mybir.ActivationFunctionType.Identity

    # input chunking: group-aligned chunks on image 0, whole-image DMAs
    # (13.3KB descs) after. img0's tail chunk is kept small so img1's
    # whole-image DMA (queued right behind it) completes before img0's
    # compute ends. NOTE: finer chunks measurably hurt — every chunk
    # boundary risks a ~2us completion-receipt stall which also drops the
    # PE out of its high p-state (425ns matmuls for the next 3us).
    # NOTE: splitting the first chunk across both queues measures WORSE —
    # the first matmul then gates on max(two noisy ~2us receipts) instead of
    # one predictable chain
    IN_CH = [[(0, 4), (4, 9), (9, 13)]] + [[(0, 13)]] * 3
    # output chunking: whole-image stores (13.3KB descs) early; img2 halves
    # so its store doesn't back-load into the tail; img3 fine-grained so the
    # final store is tiny
    OUT_CH = (
        [[(0, 13)]] * 2
        + [[(0, 7), (7, 13)]]
        + [[(0, 4), (4, 7), (7, 10), (10, 12), (12, 13)]]
    )
    # 4 blocks share each LDWEIGHTS (cuts exposed weight-load gaps) and give
    # an 8-block PSUM reuse distance so drains never stall the PE
    GROUPS = [(0, 1, 2, 3), (4, 5, 6, 7), (8, 9, 10, 11), (12,)]

    with tile.TileContext(nc) as tc:
        with (
            tc.tile_pool(name="consts", bufs=1) as consts,
            tc.tile_pool(name="xs", bufs=4) as xpool,
            tc.tile_pool(name="os", bufs=3) as opool,
            tc.tile_pool(name="ps", bufs=2, space=bass.MemorySpace.PSUM) as ppool,
        ):
            # consts ride the sync queue ahead of the inputs: the scalar
            # queue's ACT_TABLE_LOAD preamble would delay the first LDWEIGHTS
            # (hoisting the first input chunk ahead of consts measures WORSE)
            ct = consts.tile([KP, 3 * MPAD + 1], f16, tag="consts")
            nc.sync.dma_start(out=ct[:], in_=CONSTS.ap())
            smat_t = [ct[:, d * MPAD:(d + 1) * MPAD] for d in range(3)]
            bias_t = consts.tile([MP, 1], f32, tag="biasf32")
            nc.vector.tensor_copy(bias_t[:], ct[0:MP, 3 * MPAD:3 * MPAD + 1])
            bias_ap = bias_t[:, 0:1]

            # The PE runs at 1.2GHz until it has been continuously busy for
            # ~3us, only then at 2.4GHz. Warm it up on junk data while the
            # first input DMA is still in flight, so every real matmul runs
            # at full clock.
            warm = consts.tile([KP, 512], f16, tag="warm")
            nc.vector.memset(warm[:], 0.0)
            wpt = ppool.tile([MPAD, OW], f32, name="pt0")
            for _ in range(10):
                nc.tensor.matmul(
                    wpt[:], warm[:, 0:MPAD], warm[:, 1:1 + OW], start=True, stop=True
                )

            for img in range(B_PER_CORE):
                xt = xpool.tile([KP, NBLK, W], f16, name="xt")
                ot = opool.tile([MP, NBLK, OW], f16)
                out_chunks = OUT_CH[img]
                oc = 0
                loaded = 0
                in_chunks = list(IN_CH[img])
                for grp in GROUPS:
                    if grp[-1] >= loaded:
                        b0, b1 = in_chunks.pop(0)
                        nc.sync.dma_start(out=xt[:, b0:b1, :], in_=XS.ap()[img, :, b0:b1, :])
                        loaded = b1
                    pts = [
                        ppool.tile([MPAD, OW], f32, name=f"pt{i}")
                        for i in range(len(grp))
                    ]
                    for dx in range(3):
                        for pt, b in zip(pts, grp):
                            nc.tensor.matmul(
                                pt[:],
                                smat_t[dx],
                                xt[:, b, dx:dx + OW],
                                start=(dx == 0),
                                stop=(dx == 2),
                            )
                    # PSUM->SBUF drain alternates scalar/vector so neither
                    # engine's ~700ns/block copy sits on the critical path
                    for pt, b in zip(pts, grp):
                        if b % 2 == 0:
                            nc.scalar.activation(
                                ot[:, b, :], pt[0:MP, :], ident, bias=bias_ap, scale=1.0
                            )
                        else:
                            nc.vector.tensor_scalar_add(ot[:, b, :], pt[0:MP, :], bias_ap)
                        o0, o1 = out_chunks[oc]
                        if b == o1 - 1:
                            if img < 3:
                                # whole-image stores ride the gpsimd SWDGE
                                # queue: keeps desc-gen off scalar (drains)
                                # and off sync (input prefetch FIFO)
                                nc.gpsimd.dma_start(
                                    out=OUT.ap()[img, :, o0:o1, :], in_=ot[:, o0:o1, :]
                                )
                            else:
                                # img3 tail: full-width stores (all 15 DMA
                                # engines each), split across both HWDGE
                                # queues. Only (4,7) rides scalar: an issue
                                # between the LAST two drains (b10/b12, both
                                # paced by scalar) would delay the final
                                # drain that gates the final store
                                eng = nc.scalar if oc == 1 else nc.sync
                                eng.dma_start(
                                    out=OUT.ap()[img, :, o0:o1, :],
                                    in_=ot[:, o0:o1, :],
                                )
                            oc += 1

    nc.compile()
    return nc


def _get_nc():
    if "nc" not in _CACHE:
        _CACHE["nc"] = _build_program()
    return _CACHE["nc"]


def run_spmd(in_maps, **kwargs):
    from concourse.bass_utils import run_bass_kernel_spmd

    nc = _get_nc()
    return run_bass_kernel_spmd(nc, in_maps, list(range(N_CORES)), **kwargs)


def make_in_maps(X, Wc, bc):
    X = np.ascontiguousarray(np.asarray(X, dtype=np.float32))
    consts = _build_weights(Wc, bc)

    # overlap-window shard: XP[core, img, c*42+yi, b, x] = X[4*core+img, c, y0(b)+yi, x]
    # X is cast to fp16 on the host (device matmul is fp16 anyway) to halve
    # the input HBM traffic.
    Xr = X.astype(np.float16).reshape(N_CORES, B_PER_CORE, C, H, W)
    XP = np.empty((N_CORES, B_PER_CORE, C, 42, NBLK, W), dtype=np.float16)
    s = Xr.strides
    win = np.lib.stride_tricks.as_strided(
        Xr, shape=(N_CORES, B_PER_CORE, C, 12, 42, W),
        strides=(s[0], s[1], s[2], 40 * s[3], s[3], s[4]))
    XP[:, :, :, :, 0:12, :] = win.transpose(0, 1, 2, 4, 3, 5)
    XP[:, :, :, :, 12, :] = Xr[:, :, :, 470:512, :]
    XP = XP.reshape(N_CORES, B_PER_CORE, KP, NBLK, W)

    return [
        {"XS": XP[i], "CONSTS": consts}
        for i in range(N_CORES)
    ]


def gather_output(res):
    """[core][img, (c,yo), b, x] -> [32, 3, 510, 510]"""
    OUTP = np.stack([res.results[i]["OUT"] for i in range(N_CORES)]).astype(np.float32)
    R = OUTP.reshape(N_CORES, B_PER_CORE, C, 40, NBLK, OW)
    O = np.empty((N_CORES, B_PER_CORE, C, OH, OW), dtype=np.float32)
    O[:, :, :, 0:480, :] = (
        R[:, :, :, :, 0:12, :].transpose(0, 1, 2, 4, 3, 5).reshape(N_CORES, B_PER_CORE, C, 480, OW)
    )
    O[:, :, :, 480:OH, :] = R[:, :, :, 10:40, 12, :]
    return O.reshape(N_CORES * B_PER_CORE, C, OH, OW)


def kernel(X, Wc, bc, linW, linb):
    res = run_spmd(make_in_maps(X, Wc, bc))
    return gather_output(res)

